# revision 10
# baseline (speedup 1.0000x reference)
"""Multi-head attention (B=4, S=2048, D=1024, H=16) on 8 Trainium2 cores.

Sharding: DP=4 over batch x TP=2 over heads (8 heads/core). All-bf16
matmuls. Schedule is attention-paced: a short head start (K projection +
first q tile) starts the scalar-engine exp stream ~10us in; the V
projection, remaining q projections, and the output projection are woven
as PE "filler" units between attention score/PV matmuls so the tensor
engine never serializes a long projection phase against an idle scalar
engine (the baseline lost ~160us to that).

Per core:
  - K proj (sc-major, streamed x tiles), q(hp0,qp0), then blocks
    (qp, hp) qp-outer: scores S^T = kT-chunks @ q (row-tiled pairs,
    K=64), P^T = exp(S^T/8) bf16, O^T = [v | 1].T @ P^T (ones column
    fuses the softmax denominator into row 64), normalize via
    denominator broadcast DMA + reciprocal.
  - output projection partial Y_g = A_g @ Wo_g.T (bf16), spread as
    fillers through the next quarter's attention.
Host sums the two TP partials per batch and adds bo + Wo @ bv.
"""

import os
import sys

sys.path.insert(0, "/opt/trn_rl_repo")
os.environ.setdefault("MYCRO_LOCAL_CACHE", "1")

import numpy as np
import ml_dtypes
import concourse.bass as bass  # noqa: F401  (Bass types via bacc)
import concourse.mybir as mybir
import concourse.tile as tile
from concourse import bacc
from concourse.bass_utils import run_bass_kernel_spmd
from contextlib import ExitStack

f32 = mybir.dt.float32
bf16 = mybir.dt.bfloat16
AF = mybir.ActivationFunctionType
MUL = mybir.AluOpType.mult

B, S, D = 4, 2048, 1024
H = 16
DH = 64
NCORES = 8
G_HEADS = 512  # head dims per core (8 heads)


def build():
    nc = bacc.Bacc(None, target_bir_lowering=False)

    QT = nc.dram_tensor("QT", [D, S], bf16, kind="ExternalInput")
    KT = nc.dram_tensor("KT", [D, S], bf16, kind="ExternalInput")
    VT = nc.dram_tensor("VT", [D, S], bf16, kind="ExternalInput")
    WqT = nc.dram_tensor("WqT", [D, G_HEADS], bf16, kind="ExternalInput")
    WkT = nc.dram_tensor("WkT", [D, G_HEADS], bf16, kind="ExternalInput")
    WvT = nc.dram_tensor("WvT", [D, G_HEADS], bf16, kind="ExternalInput")
    WoT = nc.dram_tensor("WoT", [G_HEADS, D], bf16, kind="ExternalInput")
    bqp = nc.dram_tensor("bqp", [128, 4], f32, kind="ExternalInput")
    bkp = nc.dram_tensor("bkp", [128, 4], f32, kind="ExternalInput")
    Y = nc.dram_tensor("Y", [S, D], f32, kind="ExternalOutput")

    with tile.TileContext(nc) as tc, ExitStack() as top:
        qkpool = top.enter_context(tc.tile_pool(name="qk", bufs=1))
        vpool = top.enter_context(tc.tile_pool(name="vp", bufs=1))
        atpool = top.enter_context(tc.tile_pool(name="at", bufs=1))
        wq = top.enter_context(tc.tile_pool(name="wq", bufs=1))
        xq = top.enter_context(tc.tile_pool(name="xq", bufs=3))
        xv = top.enter_context(tc.tile_pool(name="xv", bufs=4))
        xk = top.enter_context(tc.tile_pool(name="xk", bufs=1))
        cst = top.enter_context(tc.tile_pool(name="cst", bufs=1))
        ppool = top.enter_context(tc.tile_pool(name="pP", bufs=4))
        oev = top.enter_context(tc.tile_pool(name="oev", bufs=2))
        dbp = top.enter_context(tc.tile_pool(name="dbp", bufs=2))
        osc = top.enter_context(tc.tile_pool(name="osc", bufs=2))
        yev = top.enter_context(tc.tile_pool(name="yev", bufs=3))
        drp = top.enter_context(tc.tile_pool(name="drp", bufs=4, space="DRAM"))
        pps = top.enter_context(tc.tile_pool(name="pps", bufs=1, space="PSUM"))
        spool = top.enter_context(tc.tile_pool(name="sS", bufs=2, space="PSUM"))
        opool = top.enter_context(tc.tile_pool(name="sO", bufs=3, space="PSUM"))

        # resident tensors: per-head-pair transposed layouts
        qq_t = [
            [qkpool.tile([128, 512], bf16, tag=f"qq{i}_{j}", name=f"qq{i}_{j}") for j in range(4)]
            for i in range(4)
        ]
        kT_t = [qkpool.tile([128, S], bf16, tag=f"kT{i}", name=f"kT{i}") for i in range(4)]
        v_st = [vpool.tile([128, 8 * 65], bf16, tag=f"v{i}", name=f"v{i}") for i in range(16)]
        AT_q = [
            [atpool.tile([128, 512], bf16, tag=f"AT{i}_{j}", name=f"AT{i}_{j}") for j in range(4)]
            for i in range(4)
        ]

        bq_sb = cst.tile([128, 4], f32, tag="bq")
        bk_sb = cst.tile([128, 4], f32, tag="bk")
        nc.sync.dma_start(bq_sb[:], bqp[:, :])
        nc.sync.dma_start(bk_sb[:], bkp[:, :])
        Wk_dc = [wq.tile([128, G_HEADS], bf16, tag=f"Wk{dc}", name=f"Wk{dc}") for dc in range(8)]
        Wq_dc = [wq.tile([128, G_HEADS], bf16, tag=f"Wq{dc}", name=f"Wq{dc}") for dc in range(8)]
        WvT_sb = wq.tile([128, 8, G_HEADS], bf16, tag="Wv")
        WoT_sb = wq.tile([128, 4, D], bf16, tag="Wo")
        ksrc = WkT.ap().rearrange("(d p) c -> p d c", p=128)
        qsrc = WqT.ap().rearrange("(d p) c -> p d c", p=128)
        for dc in range(8):
            nc.sync.dma_start(Wk_dc[dc][:], ksrc[:, dc, :])
        for dc in range(8):
            nc.gpsimd.dma_start(Wq_dc[dc][:], qsrc[:, dc, :])
        nc.gpsimd.dma_start(WvT_sb[:], WvT.ap().rearrange("(d p) c -> p d c", p=128))
        nc.gpsimd.dma_start(WoT_sb[:], WoT.ap().rearrange("(d p) n -> p d n", p=128))

        # warm the exp table set early (one-time ~2.7us load)
        warm = cst.tile([128, 8], f32, tag="warm")
        nc.vector.memset(warm[:], 0.0)
        nc.scalar.activation(warm[:], warm[:], AF.Exp)

        xsrc_q = QT.ap().rearrange("(d p) s -> p d s", p=128)
        xsrc_k = KT.ap().rearrange("(d p) s -> p d s", p=128)
        vsrc = VT.ap().rearrange("(d p) s -> p d s", p=128)

        # ---- projection unit emitters ----------------------------------
        def proj_unit(x_tile, W_dc, b_sb, hp, dest):
            """One [128,512] output tile of the K/Q projection: 8
            accumulating matmuls + bias add. x_tile: [128, 8, 512]."""
            ps = pps.tile([128, 512], f32, tag="ps")
            for dc in range(8):
                nc.tensor.matmul(
                    ps[:],
                    W_dc[dc][:, hp * 128 : (hp + 1) * 128],
                    x_tile[:, dc, :],
                    start=(dc == 0),
                    stop=(dc == 7),
                )
            nc.vector.tensor_scalar_add(dest, ps[:], b_sb[:, hp : hp + 1])

        def v_unit(st, xvt):
            """V-projection chunk st: [128 seq, 512 dims] + ones column."""
            ps = pps.tile([128, 512], f32, tag="ps")
            for dc in range(8):
                nc.tensor.matmul(
                    ps[:], xvt[:, dc, :], WvT_sb[:, dc, :], start=(dc == 0), stop=(dc == 7)
                )
            vd = v_st[st][:].rearrange("p (h c) -> p h c", c=65)
            nc.vector.tensor_copy(
                vd[:, :, 0:64], ps[:].rearrange("p (h c) -> p h c", c=64)
            )
            nc.vector.memset(vd[:, :, 64:65], 1.0)

        def outproj_unit(qp, q4):
            """One 128-query row block of the output projection (2 MMs
            of the N-split x 4 hp accumulation) -> Y DMA."""
            qt_g = qp * 4 + q4
            for nh in range(2):
                ps = pps.tile([128, 512], f32, tag="ps")
                for hp in range(4):
                    nc.tensor.matmul(
                        ps[:],
                        AT_q[hp][qp][:, q4 * 128 : (q4 + 1) * 128],
                        WoT_sb[:, hp, nh * 512 : (nh + 1) * 512],
                        start=(hp == 0),
                        stop=(hp == 3),
                    )
                ye = yev.tile([128, 512], f32, tag="ye")
                nc.vector.tensor_copy(ye[:], ps[:])
                nc.gpsimd.dma_start(
                    Y[qt_g * 128 : (qt_g + 1) * 128, nh * 512 : (nh + 1) * 512],
                    ye[:],
                )

        # ---- head start: K projection for (hp0, sc0) only; the rest of
        # the K projection is woven into block 0 (pre-kc hook for hp0's
        # later sc chunks, fillers for hp1-3). xk tiles stay resident.
        xk_t = [xk.tile([128, 8, 512], bf16, tag=f"xk{sc}", name=f"xk{sc}") for sc in range(4)]
        for sc in range(4):
            nc.sync.dma_start(xk_t[sc][:], xsrc_k[:, :, sc * 512 : (sc + 1) * 512])

        def k_unit(hp, sc):
            proj_unit(xk_t[sc], Wk_dc, bk_sb, hp, kT_t[hp][:, sc * 512 : (sc + 1) * 512])

        xq_t = [None] * 4
        xq_t[0] = xq.tile([128, 8, 512], bf16, tag="xt", name="xq0")
        nc.sync.dma_start(xq_t[0][:], xsrc_q[:, :, 0:512])
        for qp in (1, 2):
            xq_t[qp] = xq.tile([128, 8, 512], bf16, tag="xt", name=f"xq{qp}")
            nc.gpsimd.dma_start(xq_t[qp][:], xsrc_q[:, :, qp * 512 : (qp + 1) * 512])

        k_unit(0, 0)
        proj_unit(xq_t[0], Wq_dc, bq_sb, 0, qq_t[0][0][:])

        # prefetch first V chunks on the gpsimd queue
        xv_t = [None] * 16
        for st in range(3):
            xv_t[st] = xv.tile([128, 8, 128], bf16, tag="xv", name=f"xv{st}")
            nc.gpsimd.dma_start(xv_t[st][:], vsrc[:, :, st * 128 : (st + 1) * 128])

        # ---- filler queue ----------------------------------------------
        # Each entry is a closure emitting ~1-2us of PE work. Ordered by
        # deadline: V chunks are forced into block 0 (PV(b0,kc) consumes
        # v[kc] in FIFO order); q tiles just-in-time; outproj spread into
        # the quarter after its AT tiles complete.
        fillers = []

        def mk_v(st):
            def f():
                if st + 3 < 16:
                    xv_t[st + 3] = xv.tile(
                        [128, 8, 128], bf16, tag="xv", name=f"xv{st + 3}"
                    )
                    nc.gpsimd.dma_start(
                        xv_t[st + 3][:], vsrc[:, :, (st + 3) * 128 : (st + 4) * 128]
                    )
                v_unit(st, xv_t[st])
            return f

        def mk_q(hp, qp):
            def f():
                # prefetch the next quarter's x tile on first use of this one
                if hp == 0 and qp + 2 < 4 and xq_t[qp + 2] is None:
                    xq_t[qp + 2] = xq.tile(
                        [128, 8, 512], bf16, tag="xt", name=f"xq{qp + 2}"
                    )
                    nc.gpsimd.dma_start(
                        xq_t[qp + 2][:], xsrc_q[:, :, (qp + 2) * 512 : (qp + 3) * 512]
                    )
                proj_unit(xq_t[qp], Wq_dc, bq_sb, hp, qq_t[hp][qp][:])
            return f

        def mk_o(qp, q4):
            def f():
                outproj_unit(qp, q4)
            return f

        # q tiles needed: block (qp, hp) at index qp*4+hp needs qq[hp][qp].
        # (0,0) is done. Order remaining by first-use block index.
        qneed = sorted(
            ((qp * 4 + hp, hp, qp) for qp in range(4) for hp in range(4) if not (hp == 0 and qp == 0))
        )
        def mk_k(hp, sc):
            def f():
                k_unit(hp, sc)
            return f

        # prefix: K projection for hp 1-3 (kT[hp] needed by block hp) with
        # that hp's qp0 q tile right after; then the qp1-3 q units.
        for hp in (1, 2, 3):
            for sc in range(4):
                fillers.append(mk_k(hp, sc))
            fillers.append(mk_q(hp, 0))
        fillers.extend(
            mk_q(hp, qp) for _, hp, qp in qneed if qp != 0
        )

        # ---- attention blocks ------------------------------------------
        def emit_block(qp, hp, forced_per_kc=None, pre_kc=None):
            kt = kT_t[hp]
            qtile = qq_t[hp][qp]
            O_t = [
                opool.tile([128, 512], f32, tag="O", name="O0"),
                opool.tile([128, 512], f32, tag="O", name="O1"),
            ]
            P_prev = None
            # PV lags the exp by one kc so it never waits on the exp just
            # issued -- the PE stream stays dense. Slot 16 only drains the
            # final PV pair.
            for kc in range(17):
                if kc < 16:
                    if pre_kc is not None:
                        pre_kc(kc)
                    S_big = spool.tile([128, 1024], f32, tag="S", name="S")
                    for hloc in range(2):
                        nc.tensor.matmul(
                            S_big[:, hloc * 512 : (hloc + 1) * 512],
                            kt[hloc * 64 : hloc * 64 + 64, kc * 128 : (kc + 1) * 128],
                            qtile[hloc * 64 : hloc * 64 + 64, :],
                            start=True,
                            stop=True,
                        )
                    P_big = ppool.tile([128, 1024], bf16, tag="P", name="P")
                    nc.scalar.activation(P_big[:], S_big[:], AF.Exp, scale=0.125)
                    if forced_per_kc is not None:
                        forced_per_kc(kc)
                    if kc in (2, 5, 8, 11, 14) and fillers:
                        fillers.pop(0)()
                if kc >= 1:
                    kcv = kc - 1
                    for hloc in range(2):
                        lv = v_st[kcv][:, (2 * hp + hloc) * 65 : (2 * hp + hloc) * 65 + 65]
                        nc.tensor.matmul(
                            O_t[hloc][0:65, :],
                            lv,
                            P_prev[:, hloc * 512 : (hloc + 1) * 512],
                            start=(kcv == 0),
                            stop=(kcv == 15),
                        )
                if kc < 16:
                    P_prev = P_big
            for hloc in range(2):
                ov = oev.tile([128, 512], f32, tag="oev")
                nc.vector.tensor_copy(ov[0:65, :], O_t[hloc][0:65, :])
                dr_t = drp.tile([1, 512], f32)
                nc.sync.dma_start(dr_t[:, :], ov[64:65, :])
                db_t = dbp.tile([128, 512], f32, tag="db")
                nc.sync.dma_start(db_t[:], dr_t[0:1, :].to_broadcast([128, 512]))
                rc_t = dbp.tile([128, 512], f32, tag="rc")
                nc.vector.reciprocal_approx_fast(rc_t[:], db_t[:])
                dst = AT_q[hp][qp][hloc * 64 : hloc * 64 + 64, :]
                if hloc == 0:
                    nc.vector.tensor_tensor(dst, ov[0:64, :], rc_t[0:64, :], MUL)
                else:
                    sc_t = osc.tile([128, 512], bf16, tag="osc")
                    nc.vector.tensor_tensor(
                        sc_t[0:64, :], ov[0:64, :], rc_t[0:64, :], MUL
                    )
                    nc.sync.dma_start(dst, sc_t[0:64, :])

        vq = [mk_v(st) for st in range(16)]

        def b0_pre(kc):
            # kT[0] chunk sc must exist before scores(kc=4*sc); sc0 was
            # emitted in the head start.
            if kc in (4, 8, 12):
                k_unit(0, kc // 4)

        for qp in range(4):
            for hp in range(4):
                b = qp * 4 + hp
                if b == 0:
                    # V chunk kc emitted just before PV(b0, kc) consumes it
                    emit_block(qp, hp, lambda kc: vq[kc](), b0_pre)
                else:
                    emit_block(qp, hp)
            # after quarter qp completes, queue its outproj as fillers
            for q4 in range(4):
                fillers.append(mk_o(qp, q4))
        # drain any remaining fillers (outproj of late quarters)
        while fillers:
            fillers.pop(0)()

    nc.compile()
    return nc


_NC = None


def _get_nc():
    global _NC
    if _NC is None:
        _NC = build()
    return _NC


def _prep_core(Q, K, V, Wq, bq, Wk, bk, Wv, Wo, b, g):
    c = np.ascontiguousarray
    b16 = ml_dtypes.bfloat16
    hs = slice(g * G_HEADS, (g + 1) * G_HEADS)
    return {
        "QT": c(Q[b].T.astype(b16)),
        "KT": c(K[b].T.astype(b16)),
        "VT": c(V[b].T.astype(b16)),
        "WqT": c(Wq[hs, :].T.astype(b16)),
        "WkT": c(Wk[hs, :].T.astype(b16)),
        "WvT": c(Wv[hs, :].T.astype(b16)),
        "WoT": c(Wo[:, hs].T.astype(b16)),
        "bqp": c(bq[hs].reshape(4, 128).T),
        "bkp": c(bk[hs].reshape(4, 128).T),
    }


def kernel(Q, K, V, Wq, bq, Wk, bk, Wv, bv, Wo, bo, _want_trace=False):
    Q, K, V = (np.asarray(x, np.float32) for x in (Q, K, V))
    Wq, bq, Wk, bk, Wv, bv, Wo, bo = (
        np.asarray(x, np.float32) for x in (Wq, bq, Wk, bk, Wv, bv, Wo, bo)
    )
    nc = _get_nc()
    in_maps = [
        _prep_core(Q, K, V, Wq, bq, Wk, bk, Wv, Wo, b=c % 4, g=c // 4)
        for c in range(NCORES)
    ]
    res = run_bass_kernel_spmd(
        nc, in_maps, core_ids=list(range(NCORES)), trace=_want_trace
    )
    out = np.zeros((B, S, D), np.float32)
    for c in range(NCORES):
        out[c % 4] += res.results[c]["Y"]
    out += (bo + Wo.astype(np.float64) @ bv.astype(np.float64)).astype(np.float32)[
        None, None, :
    ]
    if _want_trace:
        kernel.last_exec_time_ns = res.exec_time_ns
        kernel.last_trace = res.instructions_and_trace
    return out


# revision 11
# speedup vs baseline: 1.0548x; 1.0548x over previous
"""Multi-head attention (B=4, S=2048, D=1024, H=16) on 8 Trainium2 cores.

Sharding: DP=4 over batch x TP=2 over heads (8 heads/core). All-bf16
matmuls. Schedule is attention-paced: a short head start (K projection +
first q tile) starts the scalar-engine exp stream ~10us in; the V
projection, remaining q projections, and the output projection are woven
as PE "filler" units between attention score/PV matmuls so the tensor
engine never serializes a long projection phase against an idle scalar
engine (the baseline lost ~160us to that).

Per core:
  - K proj (sc-major, streamed x tiles), q(hp0,qp0), then blocks
    (qp, hp) qp-outer: scores S^T = kT-chunks @ q (row-tiled pairs,
    K=64), P^T = exp(S^T/8) bf16, O^T = [v | 1].T @ P^T (ones column
    fuses the softmax denominator into row 64), normalize via
    denominator broadcast DMA + reciprocal.
  - output projection partial Y_g = A_g @ Wo_g.T (bf16), spread as
    fillers through the next quarter's attention.
Host sums the two TP partials per batch and adds bo + Wo @ bv.
"""

import os
import sys

sys.path.insert(0, "/opt/trn_rl_repo")
os.environ.setdefault("MYCRO_LOCAL_CACHE", "1")

import numpy as np
import ml_dtypes
import concourse.bass as bass  # noqa: F401  (Bass types via bacc)
import concourse.mybir as mybir
import concourse.tile as tile
from concourse import bacc
from concourse.bass_utils import run_bass_kernel_spmd
from contextlib import ExitStack

f32 = mybir.dt.float32
bf16 = mybir.dt.bfloat16
AF = mybir.ActivationFunctionType
MUL = mybir.AluOpType.mult

B, S, D = 4, 2048, 1024
H = 16
DH = 64
NCORES = 8
G_HEADS = 512  # head dims per core (8 heads)


def build():
    nc = bacc.Bacc(None, target_bir_lowering=False)

    QT = nc.dram_tensor("QT", [D, S], bf16, kind="ExternalInput")
    KT = nc.dram_tensor("KT", [D, S], bf16, kind="ExternalInput")
    VT = nc.dram_tensor("VT", [D, S], bf16, kind="ExternalInput")
    WqT = nc.dram_tensor("WqT", [D, G_HEADS], bf16, kind="ExternalInput")
    WkT = nc.dram_tensor("WkT", [D, G_HEADS], bf16, kind="ExternalInput")
    WvT = nc.dram_tensor("WvT", [D, G_HEADS], bf16, kind="ExternalInput")
    WoT = nc.dram_tensor("WoT", [G_HEADS, D], bf16, kind="ExternalInput")
    bqp = nc.dram_tensor("bqp", [128, 4], f32, kind="ExternalInput")
    bkp = nc.dram_tensor("bkp", [128, 4], f32, kind="ExternalInput")
    Y = nc.dram_tensor("Y", [S, D], f32, kind="ExternalOutput")

    with tile.TileContext(nc) as tc, ExitStack() as top:
        qkpool = top.enter_context(tc.tile_pool(name="qk", bufs=1))
        vpool = top.enter_context(tc.tile_pool(name="vp", bufs=1))
        atpool = top.enter_context(tc.tile_pool(name="at", bufs=1))
        wq = top.enter_context(tc.tile_pool(name="wq", bufs=1))
        xq = top.enter_context(tc.tile_pool(name="xq", bufs=3))
        xv = top.enter_context(tc.tile_pool(name="xv", bufs=4))
        xk = top.enter_context(tc.tile_pool(name="xk", bufs=1))
        cst = top.enter_context(tc.tile_pool(name="cst", bufs=1))
        ppool = top.enter_context(tc.tile_pool(name="pP", bufs=4))
        oev = top.enter_context(tc.tile_pool(name="oev", bufs=2))
        dbp = top.enter_context(tc.tile_pool(name="dbp", bufs=2))
        osc = top.enter_context(tc.tile_pool(name="osc", bufs=2))
        yev = top.enter_context(tc.tile_pool(name="yev", bufs=3))
        drp = top.enter_context(tc.tile_pool(name="drp", bufs=4, space="DRAM"))
        pps = top.enter_context(tc.tile_pool(name="pps", bufs=2, space="PSUM"))
        spool = top.enter_context(tc.tile_pool(name="sS", bufs=2, space="PSUM"))
        opool = top.enter_context(tc.tile_pool(name="sO", bufs=2, space="PSUM"))

        # resident tensors: per-head-pair transposed layouts
        qq_t = [
            [qkpool.tile([128, 512], bf16, tag=f"qq{i}_{j}", name=f"qq{i}_{j}") for j in range(4)]
            for i in range(4)
        ]
        kT_t = [qkpool.tile([128, S], bf16, tag=f"kT{i}", name=f"kT{i}") for i in range(4)]
        v_st = [vpool.tile([128, 8 * 65], bf16, tag=f"v{i}", name=f"v{i}") for i in range(16)]
        AT_q = [
            [atpool.tile([128, 512], bf16, tag=f"AT{i}_{j}", name=f"AT{i}_{j}") for j in range(4)]
            for i in range(4)
        ]

        bq_sb = cst.tile([128, 4], f32, tag="bq")
        bk_sb = cst.tile([128, 4], f32, tag="bk")
        nc.sync.dma_start(bq_sb[:], bqp[:, :])
        nc.sync.dma_start(bk_sb[:], bkp[:, :])
        Wk_dc = [wq.tile([128, G_HEADS], bf16, tag=f"Wk{dc}", name=f"Wk{dc}") for dc in range(8)]
        Wq_dc = [wq.tile([128, G_HEADS], bf16, tag=f"Wq{dc}", name=f"Wq{dc}") for dc in range(8)]
        WvT_sb = wq.tile([128, 8, G_HEADS], bf16, tag="Wv")
        WoT_sb = wq.tile([128, 4, D], bf16, tag="Wo")
        ksrc = WkT.ap().rearrange("(d p) c -> p d c", p=128)
        qsrc = WqT.ap().rearrange("(d p) c -> p d c", p=128)
        for dc in range(8):
            nc.sync.dma_start(Wk_dc[dc][:], ksrc[:, dc, :])
        for dc in range(8):
            nc.gpsimd.dma_start(Wq_dc[dc][:], qsrc[:, dc, :])
        nc.gpsimd.dma_start(WvT_sb[:], WvT.ap().rearrange("(d p) c -> p d c", p=128))
        nc.gpsimd.dma_start(WoT_sb[:], WoT.ap().rearrange("(d p) n -> p d n", p=128))

        # warm the exp table set early (one-time ~2.7us load)
        warm = cst.tile([128, 8], f32, tag="warm")
        nc.vector.memset(warm[:], 0.0)
        nc.scalar.activation(warm[:], warm[:], AF.Exp)

        xsrc_q = QT.ap().rearrange("(d p) s -> p d s", p=128)
        xsrc_k = KT.ap().rearrange("(d p) s -> p d s", p=128)
        vsrc = VT.ap().rearrange("(d p) s -> p d s", p=128)

        # ---- projection unit emitters ----------------------------------
        def proj_unit(x_tile, W_dc, b_sb, hp, dest):
            """One [128,512] output tile of the K/Q projection: 8
            accumulating matmuls + bias add. x_tile: [128, 8, 512]."""
            ps = pps.tile([128, 512], f32, tag="ps")
            for dc in range(8):
                nc.tensor.matmul(
                    ps[:],
                    W_dc[dc][:, hp * 128 : (hp + 1) * 128],
                    x_tile[:, dc, :],
                    start=(dc == 0),
                    stop=(dc == 7),
                )
            nc.vector.tensor_scalar_add(dest, ps[:], b_sb[:, hp : hp + 1])

        def v_unit(st, xvt):
            """V-projection chunk st: [128 seq, 512 dims] + ones column."""
            ps = pps.tile([128, 512], f32, tag="ps")
            for dc in range(8):
                nc.tensor.matmul(
                    ps[:], xvt[:, dc, :], WvT_sb[:, dc, :], start=(dc == 0), stop=(dc == 7)
                )
            vd = v_st[st][:].rearrange("p (h c) -> p h c", c=65)
            nc.vector.tensor_copy(
                vd[:, :, 0:64], ps[:].rearrange("p (h c) -> p h c", c=64)
            )
            nc.vector.memset(vd[:, :, 64:65], 1.0)

        def outproj_unit(qp, q4):
            """One 128-query row block of the output projection (2 MMs
            of the N-split x 4 hp accumulation) -> Y DMA."""
            qt_g = qp * 4 + q4
            for nh in range(2):
                ps = pps.tile([128, 512], f32, tag="ps")
                for hp in range(4):
                    nc.tensor.matmul(
                        ps[:],
                        AT_q[hp][qp][:, q4 * 128 : (q4 + 1) * 128],
                        WoT_sb[:, hp, nh * 512 : (nh + 1) * 512],
                        start=(hp == 0),
                        stop=(hp == 3),
                    )
                ye = yev.tile([128, 512], f32, tag="ye")
                nc.vector.tensor_copy(ye[:], ps[:])
                nc.gpsimd.dma_start(
                    Y[qt_g * 128 : (qt_g + 1) * 128, nh * 512 : (nh + 1) * 512],
                    ye[:],
                )

        # ---- head start: K projection for (hp0, sc0) only; the rest of
        # the K projection is woven into block 0 (pre-kc hook for hp0's
        # later sc chunks, fillers for hp1-3). xk tiles stay resident.
        xk_t = [xk.tile([128, 8, 512], bf16, tag=f"xk{sc}", name=f"xk{sc}") for sc in range(4)]
        nc.sync.dma_start(xk_t[0][:], xsrc_k[:, :, 0:512])
        xq_t = [None] * 4
        xq_t[0] = xq.tile([128, 8, 512], bf16, tag="xt", name="xq0")
        nc.sync.dma_start(xq_t[0][:], xsrc_q[:, :, 0:512])
        for sc in (1, 2, 3):
            nc.sync.dma_start(xk_t[sc][:], xsrc_k[:, :, sc * 512 : (sc + 1) * 512])

        def k_unit(hp, sc):
            proj_unit(xk_t[sc], Wk_dc, bk_sb, hp, kT_t[hp][:, sc * 512 : (sc + 1) * 512])

        for qp in (1, 2):
            xq_t[qp] = xq.tile([128, 8, 512], bf16, tag="xt", name=f"xq{qp}")
            nc.gpsimd.dma_start(xq_t[qp][:], xsrc_q[:, :, qp * 512 : (qp + 1) * 512])

        k_unit(0, 0)
        proj_unit(xq_t[0], Wq_dc, bq_sb, 0, qq_t[0][0][:])

        # prefetch first V chunks on the gpsimd queue
        xv_t = [None] * 16
        for st in range(3):
            xv_t[st] = xv.tile([128, 8, 128], bf16, tag="xv", name=f"xv{st}")
            nc.gpsimd.dma_start(xv_t[st][:], vsrc[:, :, st * 128 : (st + 1) * 128])

        # ---- filler queue ----------------------------------------------
        # Each entry is a closure emitting ~1-2us of PE work. Ordered by
        # deadline: V chunks are forced into block 0 (PV(b0,kc) consumes
        # v[kc] in FIFO order); q tiles just-in-time; outproj spread into
        # the quarter after its AT tiles complete.
        fillers = []

        def mk_v(st):
            def f():
                if st + 3 < 16:
                    xv_t[st + 3] = xv.tile(
                        [128, 8, 128], bf16, tag="xv", name=f"xv{st + 3}"
                    )
                    nc.gpsimd.dma_start(
                        xv_t[st + 3][:], vsrc[:, :, (st + 3) * 128 : (st + 4) * 128]
                    )
                v_unit(st, xv_t[st])
            return f

        def mk_q(hp, qp):
            def f():
                # prefetch the next quarter's x tile on first use of this one
                if hp == 0 and qp + 2 < 4 and xq_t[qp + 2] is None:
                    xq_t[qp + 2] = xq.tile(
                        [128, 8, 512], bf16, tag="xt", name=f"xq{qp + 2}"
                    )
                    nc.gpsimd.dma_start(
                        xq_t[qp + 2][:], xsrc_q[:, :, (qp + 2) * 512 : (qp + 3) * 512]
                    )
                proj_unit(xq_t[qp], Wq_dc, bq_sb, hp, qq_t[hp][qp][:])
            return f

        def mk_o(qp, q4):
            def f():
                outproj_unit(qp, q4)
            return f

        # q tiles needed: block (qp, hp) at index qp*4+hp needs qq[hp][qp].
        # (0,0) is done. Order remaining by first-use block index.
        qneed = sorted(
            ((qp * 4 + hp, hp, qp) for qp in range(4) for hp in range(4) if not (hp == 0 and qp == 0))
        )
        def mk_k(hp, sc):
            def f():
                k_unit(hp, sc)
            return f

        # prefix: K projection for hp 1-3 (kT[hp] needed by block hp) with
        # that hp's qp0 q tile right after; then the qp1-3 q units.
        for hp in (1, 2, 3):
            for sc in range(4):
                fillers.append(mk_k(hp, sc))
            fillers.append(mk_q(hp, 0))
        fillers.extend(
            mk_q(hp, qp) for _, hp, qp in qneed if qp != 0
        )

        # ---- attention blocks ------------------------------------------
        def emit_block(qp, hp, forced_per_kc=None, pre_kc=None):
            kt = kT_t[hp]
            qtile = qq_t[hp][qp]
            O_t = [
                opool.tile([128, 512], f32, tag="O", name="O0"),
                opool.tile([128, 512], f32, tag="O", name="O1"),
            ]
            P_prev = None
            # PV lags the exp by one kc so it never waits on the exp just
            # issued -- the PE stream stays dense. Slot 16 only drains the
            # final PV pair.
            for kc in range(17):
                if kc < 16:
                    if pre_kc is not None:
                        pre_kc(kc)
                    S_big = spool.tile([128, 1024], f32, tag="S", name="S")
                    for hloc in range(2):
                        nc.tensor.matmul(
                            S_big[:, hloc * 512 : (hloc + 1) * 512],
                            kt[hloc * 64 : hloc * 64 + 64, kc * 128 : (kc + 1) * 128],
                            qtile[hloc * 64 : hloc * 64 + 64, :],
                            start=True,
                            stop=True,
                        )
                    P_big = ppool.tile([128, 1024], bf16, tag="P", name="P")
                    nc.scalar.activation(P_big[:], S_big[:], AF.Exp, scale=0.125)
                    if forced_per_kc is not None:
                        forced_per_kc(kc)
                    if kc in (2, 5, 8, 11, 14) and fillers:
                        fillers.pop(0)()
                if kc >= 1:
                    kcv = kc - 1
                    for hloc in range(2):
                        lv = v_st[kcv][:, (2 * hp + hloc) * 65 : (2 * hp + hloc) * 65 + 65]
                        nc.tensor.matmul(
                            O_t[hloc][0:65, :],
                            lv,
                            P_prev[:, hloc * 512 : (hloc + 1) * 512],
                            start=(kcv == 0),
                            stop=(kcv == 15),
                        )
                if kc < 16:
                    P_prev = P_big
            for hloc in range(2):
                ov = oev.tile([128, 512], f32, tag="oev")
                nc.vector.tensor_copy(ov[0:65, :], O_t[hloc][0:65, :])
                dr_t = drp.tile([1, 512], f32)
                nc.sync.dma_start(dr_t[:, :], ov[64:65, :])
                db_t = dbp.tile([128, 512], f32, tag="db")
                nc.sync.dma_start(db_t[:], dr_t[0:1, :].to_broadcast([128, 512]))
                rc_t = dbp.tile([128, 512], f32, tag="rc")
                nc.vector.reciprocal_approx_fast(rc_t[:], db_t[:])
                dst = AT_q[hp][qp][hloc * 64 : hloc * 64 + 64, :]
                if hloc == 0:
                    nc.vector.tensor_tensor(dst, ov[0:64, :], rc_t[0:64, :], MUL)
                else:
                    sc_t = osc.tile([128, 512], bf16, tag="osc")
                    nc.vector.tensor_tensor(
                        sc_t[0:64, :], ov[0:64, :], rc_t[0:64, :], MUL
                    )
                    nc.sync.dma_start(dst, sc_t[0:64, :])

        vq = [mk_v(st) for st in range(16)]

        def b0_pre(kc):
            # kT[0] chunk sc must exist before scores(kc=4*sc); sc0 was
            # emitted in the head start.
            if kc in (4, 8, 12):
                k_unit(0, kc // 4)

        for qp in range(4):
            for hp in range(4):
                b = qp * 4 + hp
                if b == 0:
                    # V chunk kc emitted just before PV(b0, kc) consumes it
                    emit_block(qp, hp, lambda kc: vq[kc](), b0_pre)
                else:
                    emit_block(qp, hp)
            # after quarter qp completes, queue its outproj as fillers
            for q4 in range(4):
                fillers.append(mk_o(qp, q4))
        # drain any remaining fillers (outproj of late quarters)
        while fillers:
            fillers.pop(0)()

    nc.compile()
    return nc


_NC = None


def _get_nc():
    global _NC
    if _NC is None:
        _NC = build()
    return _NC


def _prep_core(Q, K, V, Wq, bq, Wk, bk, Wv, Wo, b, g):
    c = np.ascontiguousarray
    b16 = ml_dtypes.bfloat16
    hs = slice(g * G_HEADS, (g + 1) * G_HEADS)
    return {
        "QT": c(Q[b].T.astype(b16)),
        "KT": c(K[b].T.astype(b16)),
        "VT": c(V[b].T.astype(b16)),
        "WqT": c(Wq[hs, :].T.astype(b16)),
        "WkT": c(Wk[hs, :].T.astype(b16)),
        "WvT": c(Wv[hs, :].T.astype(b16)),
        "WoT": c(Wo[:, hs].T.astype(b16)),
        "bqp": c(bq[hs].reshape(4, 128).T),
        "bkp": c(bk[hs].reshape(4, 128).T),
    }


def kernel(Q, K, V, Wq, bq, Wk, bk, Wv, bv, Wo, bo, _want_trace=False):
    Q, K, V = (np.asarray(x, np.float32) for x in (Q, K, V))
    Wq, bq, Wk, bk, Wv, bv, Wo, bo = (
        np.asarray(x, np.float32) for x in (Wq, bq, Wk, bk, Wv, bv, Wo, bo)
    )
    nc = _get_nc()
    in_maps = [
        _prep_core(Q, K, V, Wq, bq, Wk, bk, Wv, Wo, b=c % 4, g=c // 4)
        for c in range(NCORES)
    ]
    res = run_bass_kernel_spmd(
        nc, in_maps, core_ids=list(range(NCORES)), trace=_want_trace
    )
    out = np.zeros((B, S, D), np.float32)
    for c in range(NCORES):
        out[c % 4] += res.results[c]["Y"]
    out += (bo + Wo.astype(np.float64) @ bv.astype(np.float64)).astype(np.float32)[
        None, None, :
    ]
    if _want_trace:
        kernel.last_exec_time_ns = res.exec_time_ns
        kernel.last_trace = res.instructions_and_trace
    return out


# revision 14
# speedup vs baseline: 1.0737x; 1.0179x over previous
"""Multi-head attention (B=4, S=2048, D=1024, H=16) on 8 Trainium2 cores.

Sharding: DP=4 over batch x TP=2 over heads (8 heads/core). All-bf16
matmuls. Schedule is attention-paced: a short head start (K projection +
first q tile) starts the scalar-engine exp stream ~10us in; the V
projection, remaining q projections, and the output projection are woven
as PE "filler" units between attention score/PV matmuls so the tensor
engine never serializes a long projection phase against an idle scalar
engine (the baseline lost ~160us to that).

Per core:
  - K proj (sc-major, streamed x tiles), q(hp0,qp0), then blocks
    (qp, hp) qp-outer: scores S^T = kT-chunks @ q (row-tiled pairs,
    K=64), P^T = exp(S^T/8) bf16, O^T = [v | 1].T @ P^T (ones column
    fuses the softmax denominator into row 64), normalize via
    denominator broadcast DMA + reciprocal.
  - output projection partial Y_g = A_g @ Wo_g.T (bf16), spread as
    fillers through the next quarter's attention.
Host sums the two TP partials per batch and adds bo + Wo @ bv.
"""

import os
import sys

sys.path.insert(0, "/opt/trn_rl_repo")
os.environ.setdefault("MYCRO_LOCAL_CACHE", "1")

import numpy as np
import ml_dtypes
import concourse.bass as bass  # noqa: F401  (Bass types via bacc)
import concourse.mybir as mybir
import concourse.tile as tile
from concourse import bacc
from concourse.bass_utils import run_bass_kernel_spmd
from contextlib import ExitStack

f32 = mybir.dt.float32
bf16 = mybir.dt.bfloat16
AF = mybir.ActivationFunctionType
MUL = mybir.AluOpType.mult

B, S, D = 4, 2048, 1024
H = 16
DH = 64
NCORES = 8
G_HEADS = 512  # head dims per core (8 heads)


def build():
    nc = bacc.Bacc(None, target_bir_lowering=False)

    QT = nc.dram_tensor("QT", [D, S], bf16, kind="ExternalInput")
    KT = nc.dram_tensor("KT", [D, S], bf16, kind="ExternalInput")
    VT = nc.dram_tensor("VT", [D, S], bf16, kind="ExternalInput")
    WqT = nc.dram_tensor("WqT", [D, G_HEADS], bf16, kind="ExternalInput")
    WkT = nc.dram_tensor("WkT", [D, G_HEADS], bf16, kind="ExternalInput")
    WvT = nc.dram_tensor("WvT", [D, G_HEADS], bf16, kind="ExternalInput")
    WoT = nc.dram_tensor("WoT", [G_HEADS, D], bf16, kind="ExternalInput")
    bqp = nc.dram_tensor("bqp", [128, 4], f32, kind="ExternalInput")
    bkp = nc.dram_tensor("bkp", [128, 4], f32, kind="ExternalInput")
    Y = nc.dram_tensor("Y", [S, D], f32, kind="ExternalOutput")

    with tile.TileContext(nc) as tc, ExitStack() as top:
        qkpool = top.enter_context(tc.tile_pool(name="qk", bufs=1))
        vpool = top.enter_context(tc.tile_pool(name="vp", bufs=1))
        atpool = top.enter_context(tc.tile_pool(name="at", bufs=1))
        wq = top.enter_context(tc.tile_pool(name="wq", bufs=1))
        xq = top.enter_context(tc.tile_pool(name="xq", bufs=3))
        xv = top.enter_context(tc.tile_pool(name="xv", bufs=4))
        xk = top.enter_context(tc.tile_pool(name="xk", bufs=1))
        cst = top.enter_context(tc.tile_pool(name="cst", bufs=1))
        ppool = top.enter_context(tc.tile_pool(name="pP", bufs=4))
        oev = top.enter_context(tc.tile_pool(name="oev", bufs=2))
        dbp = top.enter_context(tc.tile_pool(name="dbp", bufs=2))
        osc = top.enter_context(tc.tile_pool(name="osc", bufs=2))
        yev = top.enter_context(tc.tile_pool(name="yev", bufs=3))
        drp = top.enter_context(tc.tile_pool(name="drp", bufs=4, space="DRAM"))
        pps = top.enter_context(tc.tile_pool(name="pps", bufs=2, space="PSUM"))
        spool = top.enter_context(tc.tile_pool(name="sS", bufs=2, space="PSUM"))
        opool = top.enter_context(tc.tile_pool(name="sO", bufs=2, space="PSUM"))

        # resident tensors: per-head-pair transposed layouts
        qq_t = [
            [qkpool.tile([128, 512], bf16, tag=f"qq{i}_{j}", name=f"qq{i}_{j}") for j in range(4)]
            for i in range(4)
        ]
        kT_t = [qkpool.tile([128, S], bf16, tag=f"kT{i}", name=f"kT{i}") for i in range(4)]
        v_st = [vpool.tile([128, 8 * 65], bf16, tag=f"v{i}", name=f"v{i}") for i in range(16)]
        AT_q = [
            [atpool.tile([128, 512], bf16, tag=f"AT{i}_{j}", name=f"AT{i}_{j}") for j in range(4)]
            for i in range(4)
        ]

        bq_sb = cst.tile([128, 4], f32, tag="bq")
        bk_sb = cst.tile([128, 4], f32, tag="bk")
        nc.scalar.dma_start(bq_sb[:], bqp[:, :])
        nc.scalar.dma_start(bk_sb[:], bkp[:, :])
        Wk_dc = [wq.tile([128, G_HEADS], bf16, tag=f"Wk{dc}", name=f"Wk{dc}") for dc in range(8)]
        Wq_dc = [wq.tile([128, G_HEADS], bf16, tag=f"Wq{dc}", name=f"Wq{dc}") for dc in range(8)]
        WvT_sb = wq.tile([128, 8, G_HEADS], bf16, tag="Wv")
        WoT_sb = wq.tile([128, 4, D], bf16, tag="Wo")
        ksrc = WkT.ap().rearrange("(d p) c -> p d c", p=128)
        qsrc = WqT.ap().rearrange("(d p) c -> p d c", p=128)
        for dc in range(8):
            nc.sync.dma_start(Wk_dc[dc][:], ksrc[:, dc, :])
        nc.gpsimd.dma_start(WvT_sb[:], WvT.ap().rearrange("(d p) c -> p d c", p=128))
        nc.gpsimd.dma_start(WoT_sb[:], WoT.ap().rearrange("(d p) n -> p d n", p=128))

        # warm the exp table set early (one-time ~2.7us load)
        warm = cst.tile([128, 8], f32, tag="warm")
        nc.vector.memset(warm[:], 0.0)
        nc.scalar.activation(warm[:], warm[:], AF.Exp)

        xsrc_q = QT.ap().rearrange("(d p) s -> p d s", p=128)
        xsrc_k = KT.ap().rearrange("(d p) s -> p d s", p=128)
        vsrc = VT.ap().rearrange("(d p) s -> p d s", p=128)

        # ---- projection unit emitters ----------------------------------
        def proj_half(ps_cell, x_tile, W_dc, b_sb, hp, dest, half):
            """Half of a [128,512] K/Q projection tile (4 of 8 accumulating
            matmuls); the second half adds the bias and writes dest. The
            psum tile is carried across the two halves in ps_cell so each
            half fits the per-kc PE slack without stalling the exp stream."""
            if half == 0:
                ps_cell[0] = pps.tile([128, 512], f32, tag="ps", name="ps")
            ps = ps_cell[0]
            for dc in range(4 * half, 4 * half + 4):
                nc.tensor.matmul(
                    ps[:],
                    W_dc[dc][:, hp * 128 : (hp + 1) * 128],
                    x_tile[:, dc, :],
                    start=(dc == 0),
                    stop=(dc == 7),
                )
            if half == 1:
                nc.vector.tensor_scalar_add(dest, ps[:], b_sb[:, hp : hp + 1])

        def proj_unit(x_tile, W_dc, b_sb, hp, dest):
            cell = [None]
            proj_half(cell, x_tile, W_dc, b_sb, hp, dest, 0)
            proj_half(cell, x_tile, W_dc, b_sb, hp, dest, 1)

        def v_unit(st, xvt):
            """V-projection chunk st: [128 seq, 512 dims] + ones column."""
            ps = pps.tile([128, 512], f32, tag="ps")
            for dc in range(8):
                nc.tensor.matmul(
                    ps[:], xvt[:, dc, :], WvT_sb[:, dc, :], start=(dc == 0), stop=(dc == 7)
                )
            vd = v_st[st][:].rearrange("p (h c) -> p h c", c=65)
            nc.vector.tensor_copy(
                vd[:, :, 0:64], ps[:].rearrange("p (h c) -> p h c", c=64)
            )
            nc.vector.memset(vd[:, :, 64:65], 1.0)

        def outproj_half(qp, q4, nh):
            """Half a 128-query row block of the output projection (one
            N-split group: 4 accumulating MMs + copy + Y DMA)."""
            qt_g = qp * 4 + q4
            if True:
                ps = pps.tile([128, 512], f32, tag="ps")
                for hp in range(4):
                    nc.tensor.matmul(
                        ps[:],
                        AT_q[hp][qp][:, q4 * 128 : (q4 + 1) * 128],
                        WoT_sb[:, hp, nh * 512 : (nh + 1) * 512],
                        start=(hp == 0),
                        stop=(hp == 3),
                    )
                ye = yev.tile([128, 512], f32, tag="ye")
                nc.vector.tensor_copy(ye[:], ps[:])
                nc.gpsimd.dma_start(
                    Y[qt_g * 128 : (qt_g + 1) * 128, nh * 512 : (nh + 1) * 512],
                    ye[:],
                )

        # ---- head start: K projection for (hp0, sc0) only; the rest of
        # the K projection is woven into block 0 (pre-kc hook for hp0's
        # later sc chunks, fillers for hp1-3). xk tiles stay resident.
        xk_t = [xk.tile([128, 8, 512], bf16, tag=f"xk{sc}", name=f"xk{sc}") for sc in range(4)]
        nc.sync.dma_start(xk_t[0][:], xsrc_k[:, :, 0:512])
        xq_t = [None] * 4
        xq_t[0] = xq.tile([128, 8, 512], bf16, tag="xt", name="xq0")
        nc.sync.dma_start(xq_t[0][:], xsrc_q[:, :, 0:512])
        for dc in range(8):
            nc.sync.dma_start(Wq_dc[dc][:], qsrc[:, dc, :])
        for sc in (1, 2, 3):
            nc.sync.dma_start(xk_t[sc][:], xsrc_k[:, :, sc * 512 : (sc + 1) * 512])

        def k_unit(hp, sc):
            proj_unit(xk_t[sc], Wk_dc, bk_sb, hp, kT_t[hp][:, sc * 512 : (sc + 1) * 512])

        for qp in (1, 2):
            xq_t[qp] = xq.tile([128, 8, 512], bf16, tag="xt", name=f"xq{qp}")
            nc.gpsimd.dma_start(xq_t[qp][:], xsrc_q[:, :, qp * 512 : (qp + 1) * 512])

        k_unit(0, 0)
        proj_unit(xq_t[0], Wq_dc, bq_sb, 0, qq_t[0][0][:])

        # prefetch first V chunks on the gpsimd queue
        xv_t = [None] * 16
        for st in range(3):
            xv_t[st] = xv.tile([128, 8, 128], bf16, tag="xv", name=f"xv{st}")
            nc.gpsimd.dma_start(xv_t[st][:], vsrc[:, :, st * 128 : (st + 1) * 128])

        # ---- filler queue ----------------------------------------------
        # Each entry is a closure emitting ~1-2us of PE work. Ordered by
        # deadline: V chunks are forced into block 0 (PV(b0,kc) consumes
        # v[kc] in FIFO order); q tiles just-in-time; outproj spread into
        # the quarter after its AT tiles complete.
        fillers = []

        def mk_v(st):
            def f():
                if st + 3 < 16:
                    xv_t[st + 3] = xv.tile(
                        [128, 8, 128], bf16, tag="xv", name=f"xv{st + 3}"
                    )
                    nc.gpsimd.dma_start(
                        xv_t[st + 3][:], vsrc[:, :, (st + 3) * 128 : (st + 4) * 128]
                    )
                v_unit(st, xv_t[st])
            return f

        def mk_q(hp, qp):
            cell = [None]

            def fa():
                # prefetch the next quarter's x tile on first use of this one
                if hp == 0 and qp + 2 < 4 and xq_t[qp + 2] is None:
                    xq_t[qp + 2] = xq.tile(
                        [128, 8, 512], bf16, tag="xt", name=f"xq{qp + 2}"
                    )
                    nc.gpsimd.dma_start(
                        xq_t[qp + 2][:], xsrc_q[:, :, (qp + 2) * 512 : (qp + 3) * 512]
                    )
                proj_half(cell, xq_t[qp], Wq_dc, bq_sb, hp, qq_t[hp][qp][:], 0)

            def fb():
                proj_half(cell, xq_t[qp], Wq_dc, bq_sb, hp, qq_t[hp][qp][:], 1)

            return [fa, fb]

        def mk_o(qp, q4):
            return [lambda: outproj_half(qp, q4, 0), lambda: outproj_half(qp, q4, 1)]

        # q tiles needed: block (qp, hp) at index qp*4+hp needs qq[hp][qp].
        # (0,0) is done. Order remaining by first-use block index.
        qneed = sorted(
            ((qp * 4 + hp, hp, qp) for qp in range(4) for hp in range(4) if not (hp == 0 and qp == 0))
        )
        def mk_k(hp, sc):
            cell = [None]
            dest = kT_t[hp][:, sc * 512 : (sc + 1) * 512]
            return [
                lambda: proj_half(cell, xk_t[sc], Wk_dc, bk_sb, hp, dest, 0),
                lambda: proj_half(cell, xk_t[sc], Wk_dc, bk_sb, hp, dest, 1),
            ]

        # prefix: K projection for hp 1-3 (kT[hp] needed by block hp) with
        # that hp's qp0 q tile right after; then the qp1-3 q units.
        for hp in (1, 2, 3):
            for sc in range(4):
                fillers.extend(mk_k(hp, sc))
            fillers.extend(mk_q(hp, 0))
        for _, hp, qp in qneed:
            if qp != 0:
                fillers.extend(mk_q(hp, qp))

        # ---- attention blocks ------------------------------------------
        def emit_block(qp, hp, forced_per_kc=None, pre_kc=None):
            kt = kT_t[hp]
            qtile = qq_t[hp][qp]
            O_t = [
                opool.tile([128, 512], f32, tag="O", name="O0"),
                opool.tile([128, 512], f32, tag="O", name="O1"),
            ]
            P_prev = None
            # PV lags the exp by one kc so it never waits on the exp just
            # issued -- the PE stream stays dense. Slot 16 only drains the
            # final PV pair.
            for kc in range(17):
                if kc < 16:
                    if pre_kc is not None:
                        pre_kc(kc)
                    S_big = spool.tile([128, 1024], f32, tag="S", name="S")
                    for hloc in range(2):
                        nc.tensor.matmul(
                            S_big[:, hloc * 512 : (hloc + 1) * 512],
                            kt[hloc * 64 : hloc * 64 + 64, kc * 128 : (kc + 1) * 128],
                            qtile[hloc * 64 : hloc * 64 + 64, :],
                            start=True,
                            stop=True,
                        )
                    P_big = ppool.tile([128, 1024], bf16, tag="P", name="P")
                    nc.scalar.activation(P_big[:], S_big[:], AF.Exp, scale=0.125)
                    if forced_per_kc is not None:
                        forced_per_kc(kc)
                        if fillers:
                            fillers.pop(0)()
                    elif kc % 2 == 1 and fillers:
                        fillers.pop(0)()
                if kc >= 1:
                    kcv = kc - 1
                    for hloc in range(2):
                        lv = v_st[kcv][:, (2 * hp + hloc) * 65 : (2 * hp + hloc) * 65 + 65]
                        nc.tensor.matmul(
                            O_t[hloc][0:65, :],
                            lv,
                            P_prev[:, hloc * 512 : (hloc + 1) * 512],
                            start=(kcv == 0),
                            stop=(kcv == 15),
                        )
                if kc < 16:
                    P_prev = P_big
            for hloc in range(2):
                ov = oev.tile([128, 512], f32, tag="oev")
                nc.vector.tensor_copy(ov[0:65, :], O_t[hloc][0:65, :])
                dr_t = drp.tile([1, 512], f32)
                nc.sync.dma_start(dr_t[:, :], ov[64:65, :])
                db_t = dbp.tile([128, 512], f32, tag="db")
                nc.sync.dma_start(db_t[:], dr_t[0:1, :].to_broadcast([128, 512]))
                rc_t = dbp.tile([128, 512], f32, tag="rc")
                nc.vector.reciprocal_approx_fast(rc_t[:], db_t[:])
                dst = AT_q[hp][qp][hloc * 64 : hloc * 64 + 64, :]
                if hloc == 0:
                    nc.vector.tensor_tensor(dst, ov[0:64, :], rc_t[0:64, :], MUL)
                else:
                    sc_t = osc.tile([128, 512], bf16, tag="osc")
                    nc.vector.tensor_tensor(
                        sc_t[0:64, :], ov[0:64, :], rc_t[0:64, :], MUL
                    )
                    nc.sync.dma_start(dst, sc_t[0:64, :])

        vq = [mk_v(st) for st in range(16)]

        def b0_pre(kc):
            # kT[0] chunk sc must exist before scores(kc=4*sc); sc0 was
            # emitted in the head start.
            if kc in (4, 8, 12):
                k_unit(0, kc // 4)

        for qp in range(4):
            for hp in range(4):
                b = qp * 4 + hp
                if b == 0:
                    # V chunk kc emitted just before PV(b0, kc) consumes it
                    emit_block(qp, hp, lambda kc: vq[kc](), b0_pre)
                else:
                    emit_block(qp, hp)
            # after quarter qp completes, queue its outproj as fillers
            for q4 in range(4):
                fillers.extend(mk_o(qp, q4))
        # drain any remaining fillers (outproj of late quarters)
        while fillers:
            fillers.pop(0)()

    nc.compile()
    return nc


_NC = None


def _get_nc():
    global _NC
    if _NC is None:
        _NC = build()
    return _NC


def _prep_core(Q, K, V, Wq, bq, Wk, bk, Wv, Wo, b, g):
    c = np.ascontiguousarray
    b16 = ml_dtypes.bfloat16
    hs = slice(g * G_HEADS, (g + 1) * G_HEADS)
    return {
        "QT": c(Q[b].T.astype(b16)),
        "KT": c(K[b].T.astype(b16)),
        "VT": c(V[b].T.astype(b16)),
        "WqT": c(Wq[hs, :].T.astype(b16)),
        "WkT": c(Wk[hs, :].T.astype(b16)),
        "WvT": c(Wv[hs, :].T.astype(b16)),
        "WoT": c(Wo[:, hs].T.astype(b16)),
        "bqp": c(bq[hs].reshape(4, 128).T),
        "bkp": c(bk[hs].reshape(4, 128).T),
    }


def kernel(Q, K, V, Wq, bq, Wk, bk, Wv, bv, Wo, bo, _want_trace=False):
    Q, K, V = (np.asarray(x, np.float32) for x in (Q, K, V))
    Wq, bq, Wk, bk, Wv, bv, Wo, bo = (
        np.asarray(x, np.float32) for x in (Wq, bq, Wk, bk, Wv, bv, Wo, bo)
    )
    nc = _get_nc()
    in_maps = [
        _prep_core(Q, K, V, Wq, bq, Wk, bk, Wv, Wo, b=c % 4, g=c // 4)
        for c in range(NCORES)
    ]
    res = run_bass_kernel_spmd(
        nc, in_maps, core_ids=list(range(NCORES)), trace=_want_trace
    )
    out = np.zeros((B, S, D), np.float32)
    for c in range(NCORES):
        out[c % 4] += res.results[c]["Y"]
    out += (bo + Wo.astype(np.float64) @ bv.astype(np.float64)).astype(np.float32)[
        None, None, :
    ]
    if _want_trace:
        kernel.last_exec_time_ns = res.exec_time_ns
        kernel.last_trace = res.instructions_and_trace
    return out


# revision 15
# speedup vs baseline: 1.0940x; 1.0189x over previous
"""Multi-head attention (B=4, S=2048, D=1024, H=16) on 8 Trainium2 cores.

Sharding: DP=4 over batch x TP=2 over heads (8 heads/core). All-bf16
matmuls. Schedule is attention-paced: a short head start (K projection +
first q tile) starts the scalar-engine exp stream ~10us in; the V
projection, remaining q projections, and the output projection are woven
as PE "filler" units between attention score/PV matmuls so the tensor
engine never serializes a long projection phase against an idle scalar
engine (the baseline lost ~160us to that).

Per core:
  - K proj (sc-major, streamed x tiles), q(hp0,qp0), then blocks
    (qp, hp) qp-outer: scores S^T = kT-chunks @ q (row-tiled pairs,
    K=64), P^T = exp(S^T/8) bf16, O^T = [v | 1].T @ P^T (ones column
    fuses the softmax denominator into row 64), normalize via
    denominator broadcast DMA + reciprocal.
  - output projection partial Y_g = A_g @ Wo_g.T (bf16), spread as
    fillers through the next quarter's attention.
Host sums the two TP partials per batch and adds bo + Wo @ bv.
"""

import os
import sys

sys.path.insert(0, "/opt/trn_rl_repo")
os.environ.setdefault("MYCRO_LOCAL_CACHE", "1")

import numpy as np
import ml_dtypes
import concourse.bass as bass  # noqa: F401  (Bass types via bacc)
import concourse.mybir as mybir
import concourse.tile as tile
from concourse import bacc
from concourse.bass_utils import run_bass_kernel_spmd
from contextlib import ExitStack

f32 = mybir.dt.float32
bf16 = mybir.dt.bfloat16
AF = mybir.ActivationFunctionType
MUL = mybir.AluOpType.mult

B, S, D = 4, 2048, 1024
H = 16
DH = 64
NCORES = 8
G_HEADS = 512  # head dims per core (8 heads)


def build():
    nc = bacc.Bacc(None, target_bir_lowering=False)

    QT = nc.dram_tensor("QT", [D, S], bf16, kind="ExternalInput")
    KT = nc.dram_tensor("KT", [D, S], bf16, kind="ExternalInput")
    VT = nc.dram_tensor("VT", [D, S], bf16, kind="ExternalInput")
    WqT = nc.dram_tensor("WqT", [D, G_HEADS], bf16, kind="ExternalInput")
    WkT = nc.dram_tensor("WkT", [D, G_HEADS], bf16, kind="ExternalInput")
    WvT = nc.dram_tensor("WvT", [D, G_HEADS], bf16, kind="ExternalInput")
    WoT = nc.dram_tensor("WoT", [G_HEADS, D], bf16, kind="ExternalInput")
    bqp = nc.dram_tensor("bqp", [128, 4], f32, kind="ExternalInput")
    bkp = nc.dram_tensor("bkp", [128, 4], f32, kind="ExternalInput")
    Y = nc.dram_tensor("Y", [S, D], f32, kind="ExternalOutput")

    with tile.TileContext(nc) as tc, ExitStack() as top:
        qkpool = top.enter_context(tc.tile_pool(name="qk", bufs=1))
        vpool = top.enter_context(tc.tile_pool(name="vp", bufs=1))
        atpool = top.enter_context(tc.tile_pool(name="at", bufs=1))
        wq = top.enter_context(tc.tile_pool(name="wq", bufs=1))
        xq = top.enter_context(tc.tile_pool(name="xq", bufs=3))
        xv = top.enter_context(tc.tile_pool(name="xv", bufs=4))
        xk = top.enter_context(tc.tile_pool(name="xk", bufs=1))
        cst = top.enter_context(tc.tile_pool(name="cst", bufs=1))
        ppool = top.enter_context(tc.tile_pool(name="pP", bufs=4))
        oev = top.enter_context(tc.tile_pool(name="oev", bufs=2))
        dbp = top.enter_context(tc.tile_pool(name="dbp", bufs=2))
        osc = top.enter_context(tc.tile_pool(name="osc", bufs=2))
        yev = top.enter_context(tc.tile_pool(name="yev", bufs=3))
        drp = top.enter_context(tc.tile_pool(name="drp", bufs=4, space="DRAM"))
        pps = top.enter_context(tc.tile_pool(name="pps", bufs=2, space="PSUM"))
        spool = top.enter_context(tc.tile_pool(name="sS", bufs=2, space="PSUM"))
        opool = top.enter_context(tc.tile_pool(name="sO", bufs=2, space="PSUM"))

        # resident tensors: per-head-pair transposed layouts
        qq_t = [
            [qkpool.tile([128, 512], bf16, tag=f"qq{i}_{j}", name=f"qq{i}_{j}") for j in range(4)]
            for i in range(4)
        ]
        kT_t = [qkpool.tile([128, S], bf16, tag=f"kT{i}", name=f"kT{i}") for i in range(4)]
        v_st = [vpool.tile([128, 8 * 65], bf16, tag=f"v{i}", name=f"v{i}") for i in range(16)]
        AT_q = [
            [atpool.tile([128, 512], bf16, tag=f"AT{i}_{j}", name=f"AT{i}_{j}") for j in range(4)]
            for i in range(4)
        ]

        bq_sb = cst.tile([128, 4], f32, tag="bq")
        bk_sb = cst.tile([128, 4], f32, tag="bk")
        nc.scalar.dma_start(bq_sb[:], bqp[:, :])
        nc.scalar.dma_start(bk_sb[:], bkp[:, :])
        # (Wq/xq0 issued on the scalar queue below so the K-path preload on
        # sync and the q-path preload on scalar stream in parallel.)
        Wk_dc = [wq.tile([128, G_HEADS], bf16, tag=f"Wk{dc}", name=f"Wk{dc}") for dc in range(8)]
        Wq_dc = [wq.tile([128, G_HEADS], bf16, tag=f"Wq{dc}", name=f"Wq{dc}") for dc in range(8)]
        WvT_sb = wq.tile([128, 8, G_HEADS], bf16, tag="Wv")
        WoT_sb = wq.tile([128, 4, D], bf16, tag="Wo")
        ksrc = WkT.ap().rearrange("(d p) c -> p d c", p=128)
        qsrc = WqT.ap().rearrange("(d p) c -> p d c", p=128)
        for dc in range(8):
            nc.sync.dma_start(Wk_dc[dc][:], ksrc[:, dc, :])
        nc.gpsimd.dma_start(WvT_sb[:], WvT.ap().rearrange("(d p) c -> p d c", p=128))
        nc.gpsimd.dma_start(WoT_sb[:], WoT.ap().rearrange("(d p) n -> p d n", p=128))

        # warm the exp table set early (one-time ~2.7us load)
        warm = cst.tile([128, 8], f32, tag="warm")
        nc.vector.memset(warm[:], 0.0)
        nc.scalar.activation(warm[:], warm[:], AF.Exp)

        xsrc_q = QT.ap().rearrange("(d p) s -> p d s", p=128)
        xsrc_k = KT.ap().rearrange("(d p) s -> p d s", p=128)
        vsrc = VT.ap().rearrange("(d p) s -> p d s", p=128)

        # ---- projection unit emitters ----------------------------------
        def proj_half(ps_cell, x_tile, W_dc, b_sb, hp, dest, half):
            """Half of a [128,512] K/Q projection tile (4 of 8 accumulating
            matmuls); the second half adds the bias and writes dest. The
            psum tile is carried across the two halves in ps_cell so each
            half fits the per-kc PE slack without stalling the exp stream."""
            if half == 0:
                ps_cell[0] = pps.tile([128, 512], f32, tag="ps", name="ps")
            ps = ps_cell[0]
            for dc in range(4 * half, 4 * half + 4):
                nc.tensor.matmul(
                    ps[:],
                    W_dc[dc][:, hp * 128 : (hp + 1) * 128],
                    x_tile[:, dc, :],
                    start=(dc == 0),
                    stop=(dc == 7),
                )
            if half == 1:
                nc.vector.tensor_scalar_add(dest, ps[:], b_sb[:, hp : hp + 1])

        def proj_unit(x_tile, W_dc, b_sb, hp, dest):
            cell = [None]
            proj_half(cell, x_tile, W_dc, b_sb, hp, dest, 0)
            proj_half(cell, x_tile, W_dc, b_sb, hp, dest, 1)

        def v_unit(st, xvt):
            """V-projection chunk st: [128 seq, 512 dims] + ones column."""
            ps = pps.tile([128, 512], f32, tag="ps")
            for dc in range(8):
                nc.tensor.matmul(
                    ps[:], xvt[:, dc, :], WvT_sb[:, dc, :], start=(dc == 0), stop=(dc == 7)
                )
            vd = v_st[st][:].rearrange("p (h c) -> p h c", c=65)
            nc.vector.tensor_copy(
                vd[:, :, 0:64], ps[:].rearrange("p (h c) -> p h c", c=64)
            )
            nc.vector.memset(vd[:, :, 64:65], 1.0)

        def outproj_half(qp, q4, nh):
            """Half a 128-query row block of the output projection (one
            N-split group: 4 accumulating MMs + copy + Y DMA)."""
            qt_g = qp * 4 + q4
            if True:
                ps = pps.tile([128, 512], f32, tag="ps")
                for hp in range(4):
                    nc.tensor.matmul(
                        ps[:],
                        AT_q[hp][qp][:, q4 * 128 : (q4 + 1) * 128],
                        WoT_sb[:, hp, nh * 512 : (nh + 1) * 512],
                        start=(hp == 0),
                        stop=(hp == 3),
                    )
                ye = yev.tile([128, 512], f32, tag="ye")
                nc.vector.tensor_copy(ye[:], ps[:])
                nc.gpsimd.dma_start(
                    Y[qt_g * 128 : (qt_g + 1) * 128, nh * 512 : (nh + 1) * 512],
                    ye[:],
                )

        # ---- head start: K projection for (hp0, sc0) only; the rest of
        # the K projection is woven into block 0 (pre-kc hook for hp0's
        # later sc chunks, fillers for hp1-3). xk tiles stay resident.
        xk_t = [xk.tile([128, 8, 512], bf16, tag=f"xk{sc}", name=f"xk{sc}") for sc in range(4)]
        nc.sync.dma_start(xk_t[0][:], xsrc_k[:, :, 0:512])
        xq_t = [None] * 4
        xq_t[0] = xq.tile([128, 8, 512], bf16, tag="xt", name="xq0")
        nc.scalar.dma_start(xq_t[0][:], xsrc_q[:, :, 0:512])
        for dc in range(8):
            nc.scalar.dma_start(Wq_dc[dc][:], qsrc[:, dc, :])
        for sc in (1, 2, 3):
            nc.sync.dma_start(xk_t[sc][:], xsrc_k[:, :, sc * 512 : (sc + 1) * 512])

        def k_unit(hp, sc):
            proj_unit(xk_t[sc], Wk_dc, bk_sb, hp, kT_t[hp][:, sc * 512 : (sc + 1) * 512])

        for qp in (1, 2):
            xq_t[qp] = xq.tile([128, 8, 512], bf16, tag="xt", name=f"xq{qp}")
            nc.gpsimd.dma_start(xq_t[qp][:], xsrc_q[:, :, qp * 512 : (qp + 1) * 512])

        k_unit(0, 0)
        proj_unit(xq_t[0], Wq_dc, bq_sb, 0, qq_t[0][0][:])

        # prefetch first V chunks on the gpsimd queue
        xv_t = [None] * 16
        for st in range(3):
            xv_t[st] = xv.tile([128, 8, 128], bf16, tag="xv", name=f"xv{st}")
            nc.gpsimd.dma_start(xv_t[st][:], vsrc[:, :, st * 128 : (st + 1) * 128])

        # ---- filler queue ----------------------------------------------
        # Each entry is a closure emitting ~1-2us of PE work. Ordered by
        # deadline: V chunks are forced into block 0 (PV(b0,kc) consumes
        # v[kc] in FIFO order); q tiles just-in-time; outproj spread into
        # the quarter after its AT tiles complete.
        fillers = []

        def mk_v(st):
            def f():
                if st + 3 < 16:
                    xv_t[st + 3] = xv.tile(
                        [128, 8, 128], bf16, tag="xv", name=f"xv{st + 3}"
                    )
                    nc.gpsimd.dma_start(
                        xv_t[st + 3][:], vsrc[:, :, (st + 3) * 128 : (st + 4) * 128]
                    )
                v_unit(st, xv_t[st])
            return f

        def mk_q(hp, qp):
            cell = [None]

            def fa():
                # prefetch the next quarter's x tile on first use of this one
                if hp == 0 and qp + 2 < 4 and xq_t[qp + 2] is None:
                    xq_t[qp + 2] = xq.tile(
                        [128, 8, 512], bf16, tag="xt", name=f"xq{qp + 2}"
                    )
                    nc.gpsimd.dma_start(
                        xq_t[qp + 2][:], xsrc_q[:, :, (qp + 2) * 512 : (qp + 3) * 512]
                    )
                proj_half(cell, xq_t[qp], Wq_dc, bq_sb, hp, qq_t[hp][qp][:], 0)

            def fb():
                proj_half(cell, xq_t[qp], Wq_dc, bq_sb, hp, qq_t[hp][qp][:], 1)

            return [fa, fb]

        def mk_o(qp, q4):
            return [lambda: outproj_half(qp, q4, 0), lambda: outproj_half(qp, q4, 1)]

        # q tiles needed: block (qp, hp) at index qp*4+hp needs qq[hp][qp].
        # (0,0) is done. Order remaining by first-use block index.
        qneed = sorted(
            ((qp * 4 + hp, hp, qp) for qp in range(4) for hp in range(4) if not (hp == 0 and qp == 0))
        )
        def mk_k(hp, sc):
            cell = [None]
            dest = kT_t[hp][:, sc * 512 : (sc + 1) * 512]
            return [
                lambda: proj_half(cell, xk_t[sc], Wk_dc, bk_sb, hp, dest, 0),
                lambda: proj_half(cell, xk_t[sc], Wk_dc, bk_sb, hp, dest, 1),
            ]

        # prefix: K projection for hp 1-3 (kT[hp] needed by block hp) with
        # that hp's qp0 q tile right after; then the qp1-3 q units.
        for hp in (1, 2, 3):
            for sc in range(4):
                fillers.extend(mk_k(hp, sc))
            fillers.extend(mk_q(hp, 0))
        for _, hp, qp in qneed:
            if qp != 0:
                fillers.extend(mk_q(hp, qp))

        # ---- attention blocks ------------------------------------------
        def emit_block(qp, hp, forced_per_kc=None, pre_kc=None):
            kt = kT_t[hp]
            qtile = qq_t[hp][qp]
            O_t = [
                opool.tile([128, 512], f32, tag="O", name="O0"),
                opool.tile([128, 512], f32, tag="O", name="O1"),
            ]
            P_prev = None
            # PV lags the exp by one kc so it never waits on the exp just
            # issued -- the PE stream stays dense. Slot 16 only drains the
            # final PV pair.
            for kc in range(17):
                if kc < 16:
                    if pre_kc is not None:
                        pre_kc(kc)
                    S_big = spool.tile([128, 1024], f32, tag="S", name="S")
                    for hloc in range(2):
                        nc.tensor.matmul(
                            S_big[:, hloc * 512 : (hloc + 1) * 512],
                            kt[hloc * 64 : hloc * 64 + 64, kc * 128 : (kc + 1) * 128],
                            qtile[hloc * 64 : hloc * 64 + 64, :],
                            start=True,
                            stop=True,
                        )
                    P_big = ppool.tile([128, 1024], bf16, tag="P", name="P")
                    nc.scalar.activation(P_big[:], S_big[:], AF.Exp, scale=0.125)
                    if forced_per_kc is not None:
                        forced_per_kc(kc)
                        if fillers:
                            fillers.pop(0)()
                    elif kc % 2 == 1 and fillers:
                        fillers.pop(0)()
                if kc >= 1:
                    kcv = kc - 1
                    for hloc in range(2):
                        lv = v_st[kcv][:, (2 * hp + hloc) * 65 : (2 * hp + hloc) * 65 + 65]
                        nc.tensor.matmul(
                            O_t[hloc][0:65, :],
                            lv,
                            P_prev[:, hloc * 512 : (hloc + 1) * 512],
                            start=(kcv == 0),
                            stop=(kcv == 15),
                        )
                if kc < 16:
                    P_prev = P_big
            for hloc in (1, 0):
                ov = oev.tile([128, 512], f32, tag="oev")
                nc.vector.tensor_copy(ov[0:65, :], O_t[hloc][0:65, :])
                dr_t = drp.tile([1, 512], f32)
                nc.sync.dma_start(dr_t[:, :], ov[64:65, :])
                db_t = dbp.tile([128, 512], f32, tag="db")
                nc.sync.dma_start(db_t[:], dr_t[0:1, :].to_broadcast([128, 512]))
                rc_t = dbp.tile([128, 512], f32, tag="rc")
                nc.vector.reciprocal_approx_fast(rc_t[:], db_t[:])
                dst = AT_q[hp][qp][hloc * 64 : hloc * 64 + 64, :]
                if hloc == 0:
                    nc.vector.tensor_tensor(dst, ov[0:64, :], rc_t[0:64, :], MUL)
                else:
                    sc_t = osc.tile([128, 512], bf16, tag="osc")
                    nc.vector.tensor_tensor(
                        sc_t[0:64, :], ov[0:64, :], rc_t[0:64, :], MUL
                    )
                    nc.sync.dma_start(dst, sc_t[0:64, :])

        vq = [mk_v(st) for st in range(16)]

        def b0_pre(kc):
            # kT[0] chunk sc must exist before scores(kc=4*sc); sc0 was
            # emitted in the head start.
            if kc in (4, 8, 12):
                k_unit(0, kc // 4)

        pending_op = []
        for qp in range(4):
            for hp in range(4):
                b = qp * 4 + hp
                if b == 0:
                    # V chunk kc emitted just before PV(b0, kc) consumes it
                    emit_block(qp, hp, lambda kc: vq[kc](), b0_pre)
                else:
                    emit_block(qp, hp)
                if hp == 0 and pending_op:
                    # previous quarter's outproj becomes poppable only now:
                    # popping it during the quarter's first block would park
                    # a matmul waiting on the last normalize chain at the
                    # PE FIFO head, starving the exp stream.
                    fillers.extend(pending_op)
                    pending_op = []
            for q4 in range(4):
                pending_op.extend(mk_o(qp, q4))
        fillers.extend(pending_op)
        # drain any remaining fillers (outproj of late quarters)
        while fillers:
            fillers.pop(0)()

    nc.compile()
    return nc


_NC = None


def _get_nc():
    global _NC
    if _NC is None:
        _NC = build()
    return _NC


def _prep_core(Q, K, V, Wq, bq, Wk, bk, Wv, Wo, b, g):
    c = np.ascontiguousarray
    b16 = ml_dtypes.bfloat16
    hs = slice(g * G_HEADS, (g + 1) * G_HEADS)
    return {
        "QT": c(Q[b].T.astype(b16)),
        "KT": c(K[b].T.astype(b16)),
        "VT": c(V[b].T.astype(b16)),
        "WqT": c(Wq[hs, :].T.astype(b16)),
        "WkT": c(Wk[hs, :].T.astype(b16)),
        "WvT": c(Wv[hs, :].T.astype(b16)),
        "WoT": c(Wo[:, hs].T.astype(b16)),
        "bqp": c(bq[hs].reshape(4, 128).T),
        "bkp": c(bk[hs].reshape(4, 128).T),
    }


def kernel(Q, K, V, Wq, bq, Wk, bk, Wv, bv, Wo, bo, _want_trace=False):
    Q, K, V = (np.asarray(x, np.float32) for x in (Q, K, V))
    Wq, bq, Wk, bk, Wv, bv, Wo, bo = (
        np.asarray(x, np.float32) for x in (Wq, bq, Wk, bk, Wv, bv, Wo, bo)
    )
    nc = _get_nc()
    in_maps = [
        _prep_core(Q, K, V, Wq, bq, Wk, bk, Wv, Wo, b=c % 4, g=c // 4)
        for c in range(NCORES)
    ]
    res = run_bass_kernel_spmd(
        nc, in_maps, core_ids=list(range(NCORES)), trace=_want_trace
    )
    out = np.zeros((B, S, D), np.float32)
    for c in range(NCORES):
        out[c % 4] += res.results[c]["Y"]
    out += (bo + Wo.astype(np.float64) @ bv.astype(np.float64)).astype(np.float32)[
        None, None, :
    ]
    if _want_trace:
        kernel.last_exec_time_ns = res.exec_time_ns
        kernel.last_trace = res.instructions_and_trace
    return out
